# revision 8
# baseline (speedup 1.0000x reference)
"""Trainium2 Bass kernel for nn_EnhancedSeasonalModule (v2: channel-major fp8).

Computation (reference):
  cyc[b,s,:]   = cycle_data[(cycle_index[b]+s) % CL]
  combined     = seasonal * cyc                              [B,S,N,C]
  transformed  = combined @ W_c^T + (lin_b + b_c)            (einsum bsnc,dc->bsnd)
  z            = depthwise_conv1d_k3_same(transformed) over s, per (b,n)
  y            = gelu_exact(z + conv_b)
  ln           = layernorm_C(y) * ln_w + ln_b
  out          = seasonal + gamma * ln

Strategy: data-parallel over batch (2 of 16 per core, 8 cores). Host
pre-transposes x to [bpc, C, N, S] so every tile is channel-major [C, S]
with S contiguous per partition; the device writes out^T [bpc, C, N, S]
and the host transposes back. No PE transposes anywhere.

Per (b, node-pair): Pool computes comb = x*cyc -> fp8; PE runs the conv
as fp8 DoubleRow matmuls (taps 0+2 share one matmul via a 2-k-tile rhs
with +2-column shift into a zero-padded comb buffer; tap 1 pairs with
zero weights), plus K=1 fp8 edge-bias matmuls into the same PSUM group;
ACT applies exact GELU (uniform folded bias) -> y fp8 and squares it;
LN channel sums come from one DoubleRow stats matmul per stat per pair
(two shifted ones-windows as the two k-tiles). Per 32-tile group, DVE+ACT
turn (s1,s2) into P = gamma*ln_w_scale*rstd and Q = -mu*P rows, reshaped
to partition 0 by one SBUF DMA; PE broadcasts them as rank-1 K=1 f32r
matmuls (lnw_col x P_row) and accumulates identity@x into the Q bank;
DVE finishes out = (y*Pb) + (Q + x) in two tensor_tensor ops.

fp32r rules (hw-probed): producers feeding f32r matmuls must write
float32r-typed outputs; DMA'd operands declared float32r end-to-end;
K=1 rhs partition base must be 0/32/64 (hence the reshape-DMA), free
offset unrestricted; moving size even; PSUM out 8B-aligned.
"""

import numpy as np
from contextlib import ExitStack

import concourse.bass as bass
import concourse.bacc as bacc_mod
import concourse.tile as tile
from concourse import mybir
from concourse.bass_utils import run_bass_kernel_spmd

F32 = mybir.dt.float32
F32R = mybir.dt.float32r
FP8 = mybir.dt.float8e4
AF = mybir.ActivationFunctionType
OP = mybir.AluOpType
PM = mybir.MatmulPerfMode
FP8NP = mybir.dt.np(FP8)

B, S, N, C, CL = 16, 288, 170, 128, 24
LN_EPS = 1e-5
NCORES = 8
TB = 32  # tiles per stats group (rows of the stats PSUM bank)


def _groups(n_total, tb=TB):
    out, n0 = [], 0
    while n0 < n_total:
        t = min(tb, n_total - n0)
        out.append((n0, t))
        n0 += t
    return out


def build_program(b_per_core, n_total, s_total, gamma, use_lnb=False, repeat=1):
    nc = bacc_mod.Bacc("TRN2", target_bir_lowering=False)
    SN = s_total
    CW = 2 * SN + 4          # comb pair buffer width: [0 | t0 | 0 | 0 | t1 | 0]
    NPAIR_MAX = TB // 2

    x_d = nc.declare_dram_parameter("x", [b_per_core, C, n_total, SN], F32R, isOutput=False)
    cyct_d = nc.declare_dram_parameter("cyct", [b_per_core, C, SN], F32, isOutput=False)
    a02_d = nc.declare_dram_parameter("a02", [C, 2, C], FP8, isOutput=False)
    a1z_d = nc.declare_dram_parameter("a1z", [C, 2, C], FP8, isOutput=False)
    winp_d = nc.declare_dram_parameter("winp", [C, NPAIR_MAX, 2, TB], FP8, isOutput=False)
    ident_d = nc.declare_dram_parameter("identr", [C, C], F32R, isOutput=False)
    # rows: 0 = ln_w (or ones), 1 = gamma*ln_b (only read when use_lnb)
    lnrows_d = nc.declare_dram_parameter("lnrows", [2, C], F32R, isOutput=False)
    # edge bias rows: [0] = -lin_b*w0, [1] = -lin_b*w2 (fp8)
    ecols_d = nc.declare_dram_parameter("ecols", [2, C], FP8, isOutput=False)
    gbias_d = nc.declare_dram_parameter("gbias", [C, 1], F32, isOutput=False)
    one11_d = nc.declare_dram_parameter("one11", [1, 1], FP8, isOutput=False)
    ones288_d = nc.declare_dram_parameter("ones288", [1, SN], F32R, isOutput=False)
    out_d = nc.declare_dram_parameter("out", [b_per_core, C, n_total, SN], F32, isOutput=True)

    inv_g2 = 1.0 / (gamma * gamma)
    eps_g2 = LN_EPS * inv_g2

    with tile.TileContext(nc) as tc, ExitStack() as ctx:
        singles = ctx.enter_context(tc.tile_pool(name="singles", bufs=1))
        xin = ctx.enter_context(tc.tile_pool(name="xin", bufs=12))
        combp = ctx.enter_context(tc.tile_pool(name="combp", bufs=1))
        ypool = ctx.enter_context(tc.tile_pool(name="ypool", bufs=22))
        y2pool = ctx.enter_context(tc.tile_pool(name="y2pool", bufs=4))
        statp = ctx.enter_context(tc.tile_pool(name="statp", bufs=2))
        pqpool = ctx.enter_context(tc.tile_pool(name="pqpool", bufs=2))
        pq0pool = ctx.enter_context(tc.tile_pool(name="pq0pool", bufs=1))
        t1pool = ctx.enter_context(tc.tile_pool(name="t1pool", bufs=4))
        ostage = ctx.enter_context(tc.tile_pool(name="ostage", bufs=4))
        cycp = ctx.enter_context(tc.tile_pool(name="cycp", bufs=2))

        pz = ctx.enter_context(tc.tile_pool(name="pz", bufs=1, space="PSUM"))
        pstat = ctx.enter_context(tc.tile_pool(name="pstat", bufs=1, space="PSUM"))
        pfin = ctx.enter_context(tc.tile_pool(name="pfin", bufs=1, space="PSUM"))

        # ---- constants ----
        a02 = singles.tile([C, 2, C], FP8)
        nc.sync.dma_start(out=a02[:, :, :], in_=a02_d[:, :, :])
        a1z = singles.tile([C, 2, C], FP8)
        nc.sync.dma_start(out=a1z[:, :, :], in_=a1z_d[:, :, :])
        winp = singles.tile([C, NPAIR_MAX, 2, TB], FP8)
        nc.sync.dma_start(out=winp[:, :, :, :], in_=winp_d[:, :, :, :])
        ident = singles.tile([C, C], F32R)
        nc.sync.dma_start(out=ident[:, :], in_=ident_d[:, :])
        lnw_row = singles.tile([1, C], F32R)
        nc.sync.dma_start(out=lnw_row[:, :], in_=lnrows_d[0:1, :])
        lnbg_row = singles.tile([1, C], F32R)
        nc.sync.dma_start(out=lnbg_row[:, :], in_=lnrows_d[1:2, :])
        ecol0 = singles.tile([1, C], FP8)
        nc.sync.dma_start(out=ecol0[:, :], in_=ecols_d[0:1, :])
        ecol2 = singles.tile([1, C], FP8)
        nc.sync.dma_start(out=ecol2[:, :], in_=ecols_d[1:2, :])
        gbias = singles.tile([C, 1], F32)
        nc.sync.dma_start(out=gbias[:, :], in_=gbias_d[:, :])
        one11 = singles.tile([1, 1], FP8)
        nc.sync.dma_start(out=one11[:, :], in_=one11_d[:, :])
        ones288 = singles.tile([1, SN], F32R)
        nc.sync.dma_start(out=ones288[:, :], in_=ones288_d[:, :])
        epsg = singles.tile([TB, 1], F32)
        nc.vector.memset(epsg[:, :], eps_g2)

        # comb pair buffers: allocated once, pads stay zero across reuse
        comb_bufs = []
        for i in range(4):
            cb = combp.tile([C, CW], FP8, tag=f"cb{i}")
            nc.vector.memset(cb[:, :], 0.0)
            comb_bufs.append(cb)

        rep_ctx = tc.For_i(0, repeat, 1) if repeat > 1 else None
        if rep_ctx is not None:
            ctx.enter_context(rep_ctx)

        state = {"ci": 0}

        def emit_fin_pair(st):
            k = st["fin_i"]
            if k >= len(st["pairs"]):
                return False
            st["fin_i"] = k + 1
            n, xpair, y = st["pairs"][k]
            j0 = n - st["n0"]          # tile row within group
            b = st["b"]
            P0, Q0 = st["P0"], st["Q0"]

            pb = pfin.tile([C, 2, 512], F32, tag="pb")
            qx = pfin.tile([C, 2, 512], F32, tag="qx")
            for t in (0, 1):
                j = j0 + t
                nc.tensor.matmul(out=pb[:, t, 0:SN], lhsT=lnw_row[:, :],
                                 rhs=P0[0:1, j * SN:(j + 1) * SN],
                                 start=True, stop=True)
                nc.tensor.matmul(out=qx[:, t, 0:SN], lhsT=lnw_row[:, :],
                                 rhs=Q0[0:1, j * SN:(j + 1) * SN],
                                 start=True, stop=False)
                if use_lnb:
                    nc.tensor.matmul(out=qx[:, t, 0:SN], lhsT=lnbg_row[:, :],
                                     rhs=ones288[:, :], start=False, stop=False)
                nc.tensor.matmul(out=qx[:, t, 0:SN], lhsT=ident[:, :],
                                 rhs=xpair[:, t, :], start=False, stop=True)

            t1 = t1pool.tile([C, 2, SN], F32, tag="t1")
            nc.vector.tensor_tensor(out=t1[:, :, :], in0=y[:, :, :],
                                    in1=pb[:, :, 0:SN], op=OP.mult)
            if k % 2 == 0:
                st["ost"] = ostage.tile([C, 4, SN], F32, tag="ost", name="ost")
                st["ost_n"] = n
            half = (k % 2) * 2
            ost = st["ost"]
            nc.vector.tensor_tensor(out=ost[:, half:half + 2, :], in0=t1[:, :, :],
                                    in1=qx[:, :, 0:SN], op=OP.add)
            last = k == len(st["pairs"]) - 1
            if k % 2 == 1 or last:
                nw = half + 2
                nst = st["ost_n"]
                nc.sync.dma_start(out=out_d[b, :, nst:nst + nw, :],
                                  in_=ost[:, 0:nw, :])
            return True

        def drain(st):
            if st is None:
                return
            while emit_fin_pair(st):
                pass

        def emit_statsmath(st):
            s1 = st["st"][:, 0, 0:SN]
            s2 = st["st"][:, 1, 0:SN]
            tb = st["tb"]
            mu = statp.tile([TB, SN], F32, tag="mu")
            nc.vector.tensor_scalar_mul(out=mu[0:tb, :], in0=s1[0:tb, :], scalar1=1.0 / C)
            msq = statp.tile([TB, SN], F32, tag="msq")
            nc.vector.tensor_tensor(out=msq[0:tb, :], in0=mu[0:tb, :], in1=mu[0:tb, :], op=OP.mult)
            var = statp.tile([TB, SN], F32, tag="var")
            nc.vector.scalar_tensor_tensor(out=var[0:tb, :], in0=s2[0:tb, :],
                                           scalar=1.0 / C, in1=msq[0:tb, :],
                                           op0=OP.mult, op1=OP.subtract)
            sd = statp.tile([TB, SN], F32, tag="sd")
            nc.scalar.activation(out=sd[0:tb, :], in_=var[0:tb, :], func=AF.Sqrt,
                                 bias=epsg[0:tb, :], scale=inv_g2)
            P = pqpool.tile([TB, SN], F32R, tag="P")
            with nc.allow_low_precision(reason="float32r is bit-identical to f32"):
                nc.vector.reciprocal(out=P[0:tb, :], in_=sd[0:tb, :])
            Q = pqpool.tile([TB, SN], F32R, tag="Q")
            nc.vector.scalar_tensor_tensor(out=Q[0:tb, :], in0=mu[0:tb, :],
                                           scalar=-1.0, in1=P[0:tb, :],
                                           op0=OP.mult, op1=OP.mult)
            # reshape rows -> partition 0 so K=1 matmuls can slice by free offset
            P0 = pq0pool.tile([1, TB * SN], F32R, tag="P0")
            nc.sync.dma_start(out=P0[0:1, 0:tb * SN], in_=P[0:tb, :])
            Q0 = pq0pool.tile([1, TB * SN], F32R, tag="Q0")
            nc.sync.dma_start(out=Q0[0:1, 0:tb * SN], in_=Q[0:tb, :])
            st["P0"], st["Q0"] = P0, Q0

        pending = None
        for b in range(b_per_core):
            cyc = cycp.tile([C, SN], F32, tag="cyc")
            nc.sync.dma_start(out=cyc[:, :], in_=cyct_d[b, :, :])
            cyc_ap = bass.AP(tensor=cyc.tensor, offset=cyc.offset,
                             ap=[list(cyc.ap[0])] + [[0, 2], [1, SN]])

            for (n0, tb) in _groups(n_total):
                npairs = tb // 2
                stt = pstat.tile([TB, 2, 512], F32, tag="st")
                cur = {"b": b, "n0": n0, "tb": tb, "st": stt,
                       "pairs": [], "fin_i": 0}
                xt = None
                for pig in range(npairs):
                    n = n0 + 2 * pig
                    if pig % 2 == 0:
                        nw = min(4, tb - 2 * pig)
                        xt = xin.tile([C, 4, SN], F32R, tag="xt")
                        nc.sync.dma_start(out=xt[:, 0:nw, :],
                                          in_=x_d[b, :, n:n + nw, :])
                    xpair = xt[:, (pig % 2) * 2:(pig % 2) * 2 + 2, :]

                    cb = comb_bufs[state["ci"] % 4]
                    state["ci"] += 1
                    comb_out = bass.AP(tensor=cb.tensor, offset=cb.offset + 1,
                                       ap=[list(cb.ap[0])] + [[SN + 2, 2], [1, SN]])
                    nc.gpsimd.tensor_tensor(out=comb_out, in0=xpair.bitcast(F32),
                                            in1=cyc_ap, op=OP.mult)

                    z = pz.tile([C, 2, 512], F32, tag="z")
                    for t in (0, 1):
                        off = (SN + 2) * t
                        rhs02 = bass.AP(tensor=cb.tensor, offset=cb.offset + off,
                                        ap=[list(cb.ap[0])] + [[2, 2], [1, SN]])
                        nc.tensor.matmul(out=z[:, t, 0:SN], lhsT=a02[:, :, :],
                                         rhs=rhs02, start=True, stop=False,
                                         perf_mode=PM.DoubleRow)
                        rhs1 = bass.AP(tensor=cb.tensor, offset=cb.offset + off + 1,
                                       ap=[list(cb.ap[0])] + [[0, 2], [1, SN]])
                        nc.tensor.matmul(out=z[:, t, 0:SN], lhsT=a1z[:, :, :],
                                         rhs=rhs1, start=False, stop=False,
                                         perf_mode=PM.DoubleRow)
                        nc.tensor.matmul(out=z[:, t, 0:1], lhsT=ecol0[:, :],
                                         rhs=one11[:, :], start=False, stop=False)
                        nc.tensor.matmul(out=z[:, t, SN - 1:SN], lhsT=ecol2[:, :],
                                         rhs=one11[:, :], start=False, stop=True)

                    y = ypool.tile([C, 2, SN], FP8, tag="y")
                    nc.scalar.activation(out=y[:, :, :], in_=z[:, :, 0:SN],
                                         func=AF.Gelu, bias=gbias[:, :], scale=1.0)
                    y2 = y2pool.tile([C, 2, SN], FP8, tag="y2")
                    nc.scalar.activation(out=y2[:, :, :], in_=y[:, :, :], func=AF.Square)

                    nc.tensor.matmul(out=stt[0:tb, 0, 0:SN], lhsT=winp[:, pig, :, 0:tb],
                                     rhs=y[:, :, :], start=(pig == 0),
                                     stop=(pig == npairs - 1), perf_mode=PM.DoubleRow)
                    nc.tensor.matmul(out=stt[0:tb, 1, 0:SN], lhsT=winp[:, pig, :, 0:tb],
                                     rhs=y2[:, :, :], start=(pig == 0),
                                     stop=(pig == npairs - 1), perf_mode=PM.DoubleRow)

                    cur["pairs"].append((n, xpair, y))
                    if pending is not None:
                        emit_fin_pair(pending)

                drain(pending)
                emit_statsmath(cur)
                pending = cur

        drain(pending)

    nc.compile()
    return nc


# ------------------------- host side -------------------------

def _host_prep(inputs):
    seasonal = np.asarray(inputs["seasonal_component"], dtype=np.float32)
    cycle_index = np.asarray(inputs["cycle_index"])
    cycle_data = np.asarray(inputs["cycle_data"], dtype=np.float32)
    W_c = np.asarray(inputs["W_c"], dtype=np.float32)
    lin_b = np.asarray(inputs["lin_b"], dtype=np.float32)
    b_c = np.asarray(inputs["b_c"], dtype=np.float32)
    conv_w = np.asarray(inputs["conv_w"], dtype=np.float32)
    conv_b = np.asarray(inputs["conv_b"], dtype=np.float32)
    ln_w = np.asarray(inputs["ln_w"], dtype=np.float32)
    ln_b = np.asarray(inputs["ln_b"], dtype=np.float32)
    gamma = float(np.asarray(inputs["gamma"]))

    b_, s_, n_, c_ = seasonal.shape
    cl = cycle_data.shape[0]

    # x^T per batch: [B, C, N, S]
    xT = np.ascontiguousarray(seasonal.transpose(0, 3, 2, 1))

    idx = (cycle_index[:, None] % cl + np.arange(s_)[None, :]) % cl
    cyc = cycle_data[idx]                              # [B,S,C]
    cycT = np.ascontiguousarray(cyc.transpose(0, 2, 1))  # [B,C,S]

    w3 = conv_w[:, 0, :]                               # [C,3] taps per channel d
    lb = lin_b + b_c
    # lhsT[c, d] = W[d, c] * w3[d, k]
    a02 = np.empty((c_, 2, c_), np.float32)
    a02[:, 0, :] = W_c.T * w3[:, 0][None, :]
    a02[:, 1, :] = W_c.T * w3[:, 2][None, :]
    a1z = np.zeros((c_, 2, c_), np.float32)
    a1z[:, 0, :] = W_c.T * w3[:, 1][None, :]

    gbias = (lb * w3.sum(axis=1) + conv_b).astype(np.float32)[:, None]
    ecols = np.stack([-(lb * w3[:, 0]), -(lb * w3[:, 2])], axis=0)

    npair = TB // 2
    winp = np.zeros((c_, npair, 2, TB), np.float32)
    for p in range(npair):
        winp[:, p, 0, 2 * p] = 1.0
        winp[:, p, 1, 2 * p + 1] = 1.0

    lnrows = np.stack([ln_w, gamma * ln_b], axis=0).astype(np.float32)
    use_lnb = bool(np.any(ln_b != 0.0))

    host = {
        "xT": xT, "cycT": cycT,
        "a02": a02.astype(FP8NP), "a1z": a1z.astype(FP8NP),
        "winp": winp.astype(FP8NP), "ecols": ecols.astype(FP8NP),
        "gbias": gbias, "lnrows": lnrows,
        "identr": np.eye(c_, dtype=np.float32),
        "one11": np.ones((1, 1), FP8NP),
        "ones288": np.ones((1, s_), np.float32),
        "gamma": gamma, "use_lnb": use_lnb,
    }
    return host


_prog_cache = {}


def kernel(**inputs) -> np.ndarray:
    host = _host_prep(inputs)
    xT = host["xT"]
    b_, c_, n_, s_ = xT.shape
    bpc = b_ // NCORES

    key = (bpc, n_, s_, host["gamma"], host["use_lnb"])
    if key not in _prog_cache:
        _prog_cache[key] = build_program(
            b_per_core=bpc, n_total=n_, s_total=s_,
            gamma=host["gamma"], use_lnb=host["use_lnb"],
        )
    nc = _prog_cache[key]

    in_maps = []
    for i in range(NCORES):
        in_maps.append({
            "x": np.ascontiguousarray(xT[i * bpc:(i + 1) * bpc]),
            "cyct": np.ascontiguousarray(host["cycT"][i * bpc:(i + 1) * bpc]),
            "a02": host["a02"], "a1z": host["a1z"], "winp": host["winp"],
            "identr": host["identr"], "lnrows": host["lnrows"],
            "ecols": host["ecols"], "gbias": host["gbias"],
            "one11": host["one11"], "ones288": host["ones288"],
        })
    res = run_bass_kernel_spmd(nc, in_maps, list(range(NCORES)))
    outs = [res.results[i]["out"] for i in range(NCORES)]
    outT = np.concatenate(outs, axis=0)            # [B, C, N, S]
    return np.ascontiguousarray(outT.transpose(0, 3, 2, 1))  # [B, S, N, C]
